# revision 10
# baseline (speedup 1.0000x reference)
"""2-layer GAT on 8 Trainium2 NeuronCores.

Strategy (dst-sharded, gather-based, batched via dma_gather):
- Nodes split into 8 contiguous ranges (6250/core, padded to 6272). Each core
  owns all edges whose destination lies in its range, sorted by dst, grouped
  into 49 windows of 128 dst nodes.
- Per layer: data-parallel fused node GEMM in bf16 producing table rows
  [h(256) | al_s(H) | ones(H) | pad] with 384-col (768B) stride, plus a small
  local al_d table [npad, 64] f32. Node tables are AllGathered.
- Edge phase, per 128-dst window (nb = blocks of 128 edge slots):
    * batched src-row gather via gpsimd.dma_gather (int16 indices; edges are
      regrouped per window into src-row < 32768 and >= 32768 groups since
      indices are int16; the second gather reads a rebased table slice)
    * batched per-edge al_d gather from the local table (indices < npad)
    * w = exp(leakyrelu(al_s + al_d)): one DVE add + ACT Prelu + ACT Exp
    * htsc = [h | als | ones] * w in one bf16 DVE op; the ones columns yield
      w itself, so ONE matmul per 128-edge block accumulates both the
      weighted-message sum (cols 0:256) and the softmax denominators z
      (cols 264:272): psum += S_ed^T @ htsc
    * finalize: out = psum[:, 0:256] * recip(z); layer 1 transposes straight
      into SBUF tiles feeding the layer-2 GEMM (no DRAM round trip).
- Softmax max-subtraction dropped (cancels in alpha; logits are O(1)).
- Bias folded into the table h columns (alpha sums to 1 per destination).
"""

import numpy as np
import ml_dtypes

import concourse.bass as bass
import concourse.bacc as bacc
import concourse.tile as tile
from concourse.masks import make_identity
from concourse import mybir
from concourse.bass_utils import run_bass_kernel_spmd

BF16 = mybir.dt.bfloat16
F32 = mybir.dt.float32
I16 = mybir.dt.int16

NCORES = 8
EMB = 256
HEADS = 8
FDIM = 32
NEG_SLOPE = 0.2
DT = 384          # table row stride (cols): 256 h + 8 als + 8 ones + pad
DG = 280          # GEMM out cols: h 0:256, als 256:264, (ones via bias row
                  # 264:272), al_d 272:280
ADC = 64          # al_d table row cols (f32; 256B rows), al_d in cols 0:H
SPLIT = 32768     # int16 index limit: edges grouped by src row < / >= SPLIT
PADLOC = 300.0    # dstlocal sentinel for padded edge slots (no iota match)
USE_DMA_GATHER = False  # False: per-block indirect_dma_start fallback


def _pad128(n):
    return (n + 127) // 128 * 128


def _wrap16(idx):
    """int16 index array -> [16, ceil(n/16)] wrapped layout, tiled to 128."""
    n = len(idx)
    cols = (n + 15) // 16
    buf = np.zeros(cols * 16, dtype=np.int16)
    buf[:n] = idx
    w = np.ascontiguousarray(buf.reshape(cols, 16).T)  # element i at (i%16, i//16)
    return np.tile(w, (8, 1))


def preprocess(x, edge_index, W1, a_src1, a_dst1, b1, W2, a_src2, a_dst2, b2):
    n = x.shape[0]
    nloc = n // NCORES
    assert nloc * NCORES == n
    npad = _pad128(nloc)
    wpc = npad // 128

    src = np.concatenate([edge_index[0], np.arange(n, dtype=np.int64)]).astype(np.int64)
    dst = np.concatenate([edge_index[1], np.arange(n, dtype=np.int64)]).astype(np.int64)
    order = np.argsort(dst, kind="stable")
    src_s = src[order].astype(np.int64)
    dst_s = dst[order].astype(np.int64)
    srcrow = (src_s // nloc) * npad + (src_s % nloc)  # padded global table row

    bounds = np.searchsorted(dst_s, np.arange(NCORES + 1) * nloc)
    # per-core per-window per-group counts
    cnt1 = np.zeros((NCORES, wpc), dtype=np.int64)
    cnt2 = np.zeros((NCORES, wpc), dtype=np.int64)
    for c in range(NCORES):
        sl = slice(bounds[c], bounds[c + 1])
        dl = dst_s[sl] - c * nloc
        g2 = srcrow[sl] >= SPLIT
        cnt1[c] = np.bincount((dl // 128)[~g2], minlength=wpc)
        cnt2[c] = np.bincount((dl // 128)[g2], minlength=wpc)
    nb1 = np.maximum(1, (cnt1.max(axis=0) + 127) // 128).astype(np.int64)
    nb2 = ((cnt2.max(axis=0) + 127) // 128).astype(np.int64)
    bw = nb1 + nb2
    btot = int(bw.sum())
    woff = np.concatenate([[0], np.cumsum(bw)])  # block offset per window

    idxm = np.zeros((NCORES, 128, btot * 8), dtype=np.int16)
    idxd = np.zeros((NCORES, 128, btot * 8), dtype=np.int16)
    six = np.zeros((NCORES, btot * 128), dtype=np.int32)
    dix = np.zeros((NCORES, btot * 128), dtype=np.int32)
    dstloc = np.full((NCORES, btot * 128), PADLOC, dtype=np.float32)
    for c in range(NCORES):
        sl = slice(bounds[c], bounds[c + 1])
        sr = srcrow[sl]
        d_c = (dst_s[sl] - c * nloc).astype(np.int64)
        g2 = sr >= SPLIT
        wstart = np.searchsorted(d_c, np.arange(wpc) * 128)
        wend = np.searchsorted(d_c, np.arange(1, wpc + 1) * 128)
        for w in range(wpc):
            b0 = int(woff[w])
            for grp, nbg, boff in ((0, int(nb1[w]), b0), (1, int(nb2[w]), b0 + int(nb1[w]))):
                if nbg == 0:
                    continue
                m = slice(wstart[w], wend[w])
                sel = g2[m] if grp else ~g2[m]
                sg = sr[m][sel] - (SPLIT if grp else 0)
                dg = d_c[m][sel]
                cntg = len(sg)
                idx = np.zeros(nbg * 128, dtype=np.int16)
                idx[:cntg] = sg.astype(np.int16)
                idxm[c, :, boff * 8:(boff + nbg) * 8] = _wrap16(idx)
                dloc = np.zeros(nbg * 128, dtype=np.int16)
                dloc[:cntg] = dg.astype(np.int16)  # local dst row in [0, npad)
                idxd[c, :, boff * 8:(boff + nbg) * 8] = _wrap16(dloc)
                # dstloc in [p, b] slot layout: slot i=(j*128+p) at [p, boff+j]
                # slot-linear order i = j*128+p (dma_gather writes slot i to
                # [i%128, i//128]); the flat buffer is [b][p]-ordered, which
                # the final transpose below turns into the [p, b] layout.
                dl = np.full(nbg * 128, PADLOC, dtype=np.float32)
                dl[:cntg] = (dg - w * 128).astype(np.float32)
                dstloc[c, boff * 128:(boff + nbg) * 128] = dl
                sv = np.zeros(nbg * 128, dtype=np.int32)
                sv[:cntg] = (sg.astype(np.int64) + (SPLIT if grp else 0)).astype(np.int32)
                six[c, boff * 128:(boff + nbg) * 128] = sv
                dv = np.zeros(nbg * 128, dtype=np.int32)
                dv[:cntg] = dg.astype(np.int32)
                dix[c, boff * 128:(boff + nbg) * 128] = dv
    dstloc = np.ascontiguousarray(
        dstloc.reshape(NCORES, btot, 128).transpose(0, 2, 1)).astype(np.float32)
    six = np.ascontiguousarray(six.reshape(NCORES, btot, 128).transpose(0, 2, 1))
    dix = np.ascontiguousarray(dix.reshape(NCORES, btot, 128).transpose(0, 2, 1))

    # fused GEMM weights, head-interleaved columns c = f*HEADS + h
    W1f = W1.reshape(EMB, HEADS, FDIM).transpose(0, 2, 1).reshape(EMB, EMB)
    A_s1 = np.einsum("dhf,hf->dh", W1.reshape(EMB, HEADS, FDIM), a_src1)
    A_d1 = np.einsum("dhf,hf->dh", W1.reshape(EMB, HEADS, FDIM), a_dst1)
    wc1 = np.zeros((EMB, DG), dtype=np.float32)
    wc1[:, :EMB] = W1f
    wc1[:, EMB:EMB + HEADS] = A_s1
    wc1[:, 272:280] = A_d1
    brow1 = np.zeros((DG,), dtype=np.float32)
    brow1[:EMB] = b1.reshape(HEADS, FDIM).T.reshape(EMB)  # interleaved
    brow1[264:272] = 1.0  # ones columns -> z via the aggregation matmul

    c_idx = np.arange(EMB)
    perm = (c_idx % HEADS) * FDIM + (c_idx // HEADS)
    W2p = W2.reshape(EMB, EMB)[perm, :]
    wc2 = np.zeros((EMB, DG), dtype=np.float32)
    wc2[:, :EMB] = W2p
    wc2[:, EMB] = W2p @ a_src2.reshape(EMB)
    wc2[:, 272] = W2p @ a_dst2.reshape(EMB)
    brow2 = np.zeros((DG,), dtype=np.float32)
    brow2[:EMB] = b2
    brow2[257] = 1.0  # ones column for layer-2 z

    xsT = np.zeros((NCORES, EMB, npad), dtype=np.float32)
    for c in range(NCORES):
        xsT[c, :, :nloc] = x[c * nloc:(c + 1) * nloc].T

    common = {
        "wc1": wc1.astype(ml_dtypes.bfloat16),
        "wc2": wc2.astype(ml_dtypes.bfloat16),
        "brow1": brow1.astype(ml_dtypes.bfloat16)[None, :],
        "brow2": brow2.astype(ml_dtypes.bfloat16)[None, :],
    }
    in_maps = []
    for c in range(NCORES):
        in_maps.append(dict(common,
                            xsT=xsT[c].astype(ml_dtypes.bfloat16),
                            idxm=idxm[c], idxd=idxd[c],
                            six=six[c], dix=dix[c],
                            dstloc=dstloc[c]))
    meta = dict(n=n, nloc=nloc, npad=npad, wpc=wpc,
                nb1=[int(v) for v in nb1], nb2=[int(v) for v in nb2],
                btot=btot)
    return in_maps, meta


def build(meta):
    npad, wpc, btot = meta["npad"], meta["wpc"], meta["btot"]
    nb1, nb2 = meta["nb1"], meta["nb2"]
    ntot = npad * NCORES
    nc = bacc.Bacc("TRN2", target_bir_lowering=False, debug=False, num_devices=NCORES)

    xsT = nc.dram_tensor("xsT", [EMB, npad], BF16, kind="ExternalInput")
    wc1 = nc.dram_tensor("wc1", [EMB, DG], BF16, kind="ExternalInput")
    wc2 = nc.dram_tensor("wc2", [EMB, DG], BF16, kind="ExternalInput")
    brow1 = nc.dram_tensor("brow1", [1, DG], BF16, kind="ExternalInput")
    brow2 = nc.dram_tensor("brow2", [1, DG], BF16, kind="ExternalInput")
    idxm = nc.dram_tensor("idxm", [128, btot * 8], I16, kind="ExternalInput")
    idxd = nc.dram_tensor("idxd", [128, btot * 8], I16, kind="ExternalInput")
    dstloc = nc.dram_tensor("dstloc", [128, btot], F32, kind="ExternalInput")
    six = nc.dram_tensor("six", [128, btot], mybir.dt.int32, kind="ExternalInput")
    dix = nc.dram_tensor("dix", [128, btot], mybir.dt.int32, kind="ExternalInput")
    out = nc.dram_tensor("out", [npad, EMB], F32, kind="ExternalOutput")

    t1loc = nc.dram_tensor("t1loc", [npad, DT], BF16)
    t2loc = nc.dram_tensor("t2loc", [npad, DT], BF16)
    ald1 = nc.dram_tensor("ald1", [npad, ADC], F32)
    ald2 = nc.dram_tensor("ald2", [npad, ADC], F32)
    t1 = nc.dram_tensor("t1", [ntot, DT], BF16, addr_space="Shared")
    t2 = nc.dram_tensor("t2", [ntot, DT], BF16, addr_space="Shared")

    rg = [list(range(NCORES))]

    with tile.TileContext(nc) as tc:
        with (
            tc.tile_pool(name="const", bufs=1) as constp,
            tc.tile_pool(name="psum", bufs=2, space="PSUM") as psump,
        ):
            iota_i = constp.tile([128, 128], mybir.dt.int32)
            nc.gpsimd.iota(iota_i[:], pattern=[[1, 128]], base=0, channel_multiplier=0)
            iota128 = constp.tile([128, 128], BF16)
            nc.vector.tensor_copy(out=iota128[:], in_=iota_i[:])
            ones_row = constp.tile([1, 128], BF16)
            nc.vector.memset(ones_row[:], 1.0)
            ident = constp.tile([128, 128], BF16)
            make_identity(nc, ident[:])

            idxm_t = constp.tile([128, btot * 8], I16)
            nc.sync.dma_start(out=idxm_t[:], in_=idxm[:])
            idxd_t = constp.tile([128, btot * 8], I16)
            nc.sync.dma_start(out=idxd_t[:], in_=idxd[:])
            dstloc_t = constp.tile([128, btot], F32)
            nc.sync.dma_start(out=dstloc_t[:], in_=dstloc[:])
            six_t = constp.tile([128, btot], mybir.dt.int32)
            nc.sync.dma_start(out=six_t[:], in_=six[:])
            dix_t = constp.tile([128, btot], mybir.dt.int32)
            nc.sync.dma_start(out=dix_t[:], in_=dix[:])

            o1T_0 = constp.tile([128, npad], BF16, tag="o1T0")
            o1T_1 = constp.tile([128, npad], BF16, tag="o1T1")
            o1T = [o1T_0, o1T_1]

            def node_gemm(wc_dram, brow_dram, tdst, ald_dst, heads, src_tiles=None):
                with tc.tile_pool(name="gemm", bufs=2) as gp:
                    wck = []
                    for k in range(2):
                        t = gp.tile([128, DG], BF16, tag=f"wc{k}")
                        nc.sync.dma_start(out=t[:], in_=wc_dram[k * 128:(k + 1) * 128, :])
                        wck.append(t)
                    br = gp.tile([1, DG], BF16, tag="brow")
                    nc.sync.dma_start(out=br[:], in_=brow_dram[:])
                    if src_tiles is None:
                        xk = []
                        for k in range(2):
                            t = gp.tile([128, npad], BF16, tag=f"x{k}")
                            nc.sync.dma_start(out=t[:], in_=xsT[k * 128:(k + 1) * 128, :])
                            xk.append(t)
                    else:
                        xk = src_tiles
                    for m in range(npad // 128):
                        ps = psump.tile([128, DG], F32, tag="gemm_ps", bufs=2)
                        sl = slice(m * 128, (m + 1) * 128)
                        nc.tensor.matmul(ps[:], lhsT=xk[0][:, sl], rhs=wck[0][:], start=True, stop=False)
                        nc.tensor.matmul(ps[:], lhsT=xk[1][:, sl], rhs=wck[1][:], start=False, stop=False)
                        nc.tensor.matmul(ps[:], lhsT=ones_row[:], rhs=br[:], start=False, stop=True)
                        ot = gp.tile([128, 272], BF16, tag="gemm_out", bufs=3)
                        nc.scalar.copy(out=ot[:], in_=ps[:, 0:272])
                        nc.sync.dma_start(out=tdst[sl, 0:272], in_=ot[:])
                        at = gp.tile([128, heads], F32, tag="gemm_ald", bufs=3)
                        nc.scalar.copy(out=at[:], in_=ps[:, 272:272 + heads])
                        nc.sync.dma_start(out=ald_dst[sl, 0:heads], in_=at[:])

            def edge_phase(tbl, ald_dram, heads, writer):
                fexp = 272 // 8 if heads > 1 else 0  # 34 groups of 8 cols (L1)
                zc0 = 264 if heads > 1 else 257      # z column offset in psum
                rw = 272 if heads > 1 else 258       # agg matmul rhs width
                with tc.tile_pool(name="edge", bufs=2) as ep:
                    b0 = 0
                    for w in range(wpc):
                        m1, m2 = nb1[w], nb2[w]
                        nb = m1 + m2
                        ht = ep.tile([128, nb * DT], BF16, tag="ht", bufs=2)
                        htv = ht[:].rearrange("p (j c) -> p j c", c=DT)
                        alde = ep.tile([128, nb * ADC], F32, tag="alde", bufs=2)
                        aldev = alde[:].rearrange("p (j c) -> p j c", c=ADC)
                        if USE_DMA_GATHER:
                            nc.gpsimd.dma_gather(
                                out_ap=htv[:, 0:m1, :], in_ap=tbl[0:SPLIT, :],
                                idxs_ap=idxm_t[:, b0 * 8:(b0 + m1) * 8],
                                num_idxs=m1 * 128, num_idxs_reg=m1 * 128,
                                elem_size=DT)
                            if m2 > 0:
                                nc.gpsimd.dma_gather(
                                    out_ap=htv[:, m1:nb, :],
                                    in_ap=tbl[SPLIT:ntot, :],
                                    idxs_ap=idxm_t[:, (b0 + m1) * 8:(b0 + nb) * 8],
                                    num_idxs=m2 * 128, num_idxs_reg=m2 * 128,
                                    elem_size=DT)
                            nc.gpsimd.dma_gather(
                                out_ap=aldev[:, :, :], in_ap=ald_dram[:],
                                idxs_ap=idxd_t[:, b0 * 8:(b0 + nb) * 8],
                                num_idxs=nb * 128, num_idxs_reg=nb * 128,
                                elem_size=ADC)
                        else:
                            for j in range(nb):
                                nc.gpsimd.indirect_dma_start(
                                    out=htv[:, j, 0:272], out_offset=None, in_=tbl[:],
                                    in_offset=bass.IndirectOffsetOnAxis(
                                        ap=six_t[:, b0 + j:b0 + j + 1], axis=0))
                                nc.gpsimd.indirect_dma_start(
                                    out=aldev[:, j, 0:heads], out_offset=None, in_=ald_dram[:],
                                    in_offset=bass.IndirectOffsetOnAxis(
                                        ap=dix_t[:, b0 + j:b0 + j + 1], axis=0))
                        wpre = ep.tile([128, nb * heads], F32, tag="wpre", bufs=2)
                        nc.vector.tensor_add(
                            out=wpre[:].rearrange("p (j h) -> p j h", h=heads),
                            in0=htv[:, :, EMB:EMB + heads],
                            in1=aldev[:, :, 0:heads])
                        wlr = ep.tile([128, nb * heads], F32, tag="wlr", bufs=2)
                        nc.scalar.activation(out=wlr[:], in_=wpre[:],
                                             func=mybir.ActivationFunctionType.Prelu,
                                             alpha=NEG_SLOPE)
                        wwin = ep.tile([128, nb * heads], BF16 if heads > 1 else F32,
                                       tag="wwin", bufs=2)
                        nc.scalar.activation(out=wwin[:], in_=wlr[:],
                                             func=mybir.ActivationFunctionType.Exp)
                        if heads > 1:
                            htsc = ep.tile([128, nb * 272], BF16, tag="htsc", bufs=2)
                            nc.vector.tensor_mul(
                                out=htsc[:].rearrange("p (j f h) -> p j f h", f=fexp, h=heads),
                                in0=htv[:, :, 0:272].rearrange("p j (f h) -> p j f h", h=heads),
                                in1=wwin[:].rearrange("p (j h) -> p j h", h=heads)
                                    .unsqueeze(2).to_broadcast([128, nb, fexp, heads]))
                        ps = psump.tile([128, rw], F32, tag="agg")
                        for j in range(nb):
                            b = b0 + j
                            s_ed = ep.tile([128, 128], BF16, tag="s_ed", bufs=4)
                            if heads > 1:
                                nc.vector.tensor_scalar(
                                    out=s_ed[:], in0=iota128[:],
                                    scalar1=dstloc_t[:, b:b + 1], scalar2=None,
                                    op0=mybir.AluOpType.is_equal)
                                rhs = htsc[:, j * 272:(j + 1) * 272]
                            else:
                                nc.vector.tensor_scalar(
                                    out=s_ed[:], in0=iota128[:],
                                    scalar1=dstloc_t[:, b:b + 1],
                                    scalar2=wwin[:, j:j + 1],
                                    op0=mybir.AluOpType.is_equal,
                                    op1=mybir.AluOpType.mult)
                                rhs = ht[:, j * DT:j * DT + rw]
                            nc.tensor.matmul(ps[:], lhsT=s_ed[:], rhs=rhs,
                                             start=(j == 0), stop=(j == nb - 1))
                        zn = heads
                        zeps = ep.tile([128, zn], F32, tag="zeps", bufs=2)
                        nc.vector.tensor_scalar_add(out=zeps[:], in0=ps[:, zc0:zc0 + zn],
                                                    scalar1=1e-16)
                        rz = ep.tile([128, zn], F32, tag="rz", bufs=2)
                        nc.vector.reciprocal(out=rz[:], in_=zeps[:])
                        writer(w, ps, rz, ep)
                        b0 += nb

            # ---- layer 1 ----
            node_gemm(wc1, brow1, t1loc, ald1, HEADS)
            nc.gpsimd.collective_compute(
                "AllGather", mybir.AluOpType.bypass, replica_groups=rg,
                ins=[t1loc[:]], outs=[t1[:]])

            def write1(w, ps, rz, ep):
                ot = ep.tile([128, EMB], BF16, tag="outw", bufs=2)
                nc.vector.tensor_mul(
                    out=ot[:].rearrange("p (f h) -> p f h", h=HEADS),
                    in0=ps[:, 0:EMB].rearrange("p (f h) -> p f h", h=HEADS),
                    in1=rz[:].unsqueeze(1).to_broadcast([128, FDIM, HEADS]))
                for k in range(2):
                    pst = psump.tile([128, 128], BF16, tag="tr_ps")
                    nc.tensor.transpose(out=pst[:], in_=ot[:, k * 128:(k + 1) * 128],
                                        identity=ident[:])
                    nc.scalar.copy(out=o1T[k][:, w * 128:(w + 1) * 128], in_=pst[:])

            edge_phase(t1, ald1, HEADS, write1)

            # ---- layer 2 ----
            node_gemm(wc2, brow2, t2loc, ald2, 1, src_tiles=o1T)
            nc.gpsimd.collective_compute(
                "AllGather", mybir.AluOpType.bypass, replica_groups=rg,
                ins=[t2loc[:]], outs=[t2[:]])

            def write2(w, ps, rz, ep):
                ot = ep.tile([128, EMB], F32, tag="outw2", bufs=2)
                nc.vector.tensor_mul(
                    out=ot[:], in0=ps[:, 0:EMB],
                    in1=rz[:, 0:1].to_broadcast([128, EMB]))
                nc.sync.dma_start(out=out[w * 128:(w + 1) * 128, :], in_=ot[:])

            edge_phase(t2, ald2, 1, write2)

    nc.compile()
    return nc


def kernel(**inputs):
    inputs = {k: np.asarray(v) for k, v in inputs.items()}
    in_maps, meta = preprocess(**inputs)
    nc = build(meta)
    res = run_bass_kernel_spmd(nc, in_maps, list(range(NCORES)))
    nloc = meta["nloc"]
    parts = [res.results[c]["out"][:nloc] for c in range(NCORES)]
    return np.concatenate(parts, axis=0).astype(np.float32)


# revision 11
# speedup vs baseline: 1.3709x; 1.3709x over previous
"""2-layer GAT on 8 Trainium2 NeuronCores.

Strategy (dst-sharded, gather-based, batched via dma_gather):
- Nodes split into 8 contiguous ranges (6250/core, padded to 6272). Each core
  owns all edges whose destination lies in its range, sorted by dst, grouped
  into 49 windows of 128 dst nodes.
- Per layer: data-parallel fused node GEMM in bf16 producing table rows
  [h(256) | al_s(H) | ones(H) | pad] with 384-col (768B) stride, plus a small
  local al_d table [npad, 64] f32. Node tables are AllGathered.
- Edge phase, per 128-dst window (nb = blocks of 128 edge slots):
    * batched src-row gather via gpsimd.dma_gather (int16 indices; edges are
      regrouped per window into src-row < 32768 and >= 32768 groups since
      indices are int16; the second gather reads a rebased table slice)
    * batched per-edge al_d gather from the local table (indices < npad)
    * w = exp(leakyrelu(al_s + al_d)): one DVE add + ACT Prelu + ACT Exp
    * htsc = [h | als | ones] * w in one bf16 DVE op; the ones columns yield
      w itself, so ONE matmul per 128-edge block accumulates both the
      weighted-message sum (cols 0:256) and the softmax denominators z
      (cols 264:272): psum += S_ed^T @ htsc
    * finalize: out = psum[:, 0:256] * recip(z); layer 1 transposes straight
      into SBUF tiles feeding the layer-2 GEMM (no DRAM round trip).
- Softmax max-subtraction dropped (cancels in alpha; logits are O(1)).
- Bias folded into the table h columns (alpha sums to 1 per destination).
"""

import numpy as np
import ml_dtypes

import concourse.bass as bass
import concourse.bacc as bacc
import concourse.tile as tile
from concourse.masks import make_identity
from concourse import mybir
from concourse.bass_utils import run_bass_kernel_spmd

BF16 = mybir.dt.bfloat16
F32 = mybir.dt.float32
I16 = mybir.dt.int16

NCORES = 8
EMB = 256
HEADS = 8
FDIM = 32
NEG_SLOPE = 0.2
DT = 384          # table row stride (cols): 256 h + 8 als + 8 ones + pad
DG = 280          # GEMM out cols: h 0:256, als 256:264, (ones via bias row
                  # 264:272), al_d 272:280
ADC = 64          # al_d table row cols (f32; 256B rows), al_d in cols 0:H
SPLIT = 32768     # int16 index limit: edges grouped by src row < / >= SPLIT
PADLOC = 300.0    # dstlocal sentinel for padded edge slots (no iota match)
USE_DMA_GATHER = True  # False: per-block indirect_dma_start fallback


def _pad128(n):
    return (n + 127) // 128 * 128


def _wrap16(idx):
    """int16 index array -> [16, ceil(n/16)] wrapped layout, tiled to 128."""
    n = len(idx)
    cols = (n + 15) // 16
    buf = np.zeros(cols * 16, dtype=np.int16)
    buf[:n] = idx
    w = np.ascontiguousarray(buf.reshape(cols, 16).T)  # element i at (i%16, i//16)
    return np.tile(w, (8, 1))


def preprocess(x, edge_index, W1, a_src1, a_dst1, b1, W2, a_src2, a_dst2, b2):
    n = x.shape[0]
    nloc = n // NCORES
    assert nloc * NCORES == n
    npad = _pad128(nloc)
    wpc = npad // 128

    src = np.concatenate([edge_index[0], np.arange(n, dtype=np.int64)]).astype(np.int64)
    dst = np.concatenate([edge_index[1], np.arange(n, dtype=np.int64)]).astype(np.int64)
    order = np.argsort(dst, kind="stable")
    src_s = src[order].astype(np.int64)
    dst_s = dst[order].astype(np.int64)
    srcrow = (src_s // nloc) * npad + (src_s % nloc)  # padded global table row

    bounds = np.searchsorted(dst_s, np.arange(NCORES + 1) * nloc)
    # per-core per-window per-group counts
    cnt1 = np.zeros((NCORES, wpc), dtype=np.int64)
    cnt2 = np.zeros((NCORES, wpc), dtype=np.int64)
    for c in range(NCORES):
        sl = slice(bounds[c], bounds[c + 1])
        dl = dst_s[sl] - c * nloc
        g2 = srcrow[sl] >= SPLIT
        cnt1[c] = np.bincount((dl // 128)[~g2], minlength=wpc)
        cnt2[c] = np.bincount((dl // 128)[g2], minlength=wpc)
    nb1 = np.maximum(1, (cnt1.max(axis=0) + 127) // 128).astype(np.int64)
    nb2 = ((cnt2.max(axis=0) + 127) // 128).astype(np.int64)
    bw = nb1 + nb2
    btot = int(bw.sum())
    woff = np.concatenate([[0], np.cumsum(bw)])  # block offset per window

    idxm = np.zeros((NCORES, 128, btot * 8), dtype=np.int16)
    idxd = np.zeros((NCORES, 128, btot * 8), dtype=np.int16)
    six = np.zeros((NCORES, btot * 128), dtype=np.int32)
    dix = np.zeros((NCORES, btot * 128), dtype=np.int32)
    dstloc = np.full((NCORES, btot * 128), PADLOC, dtype=np.float32)
    for c in range(NCORES):
        sl = slice(bounds[c], bounds[c + 1])
        sr = srcrow[sl]
        d_c = (dst_s[sl] - c * nloc).astype(np.int64)
        g2 = sr >= SPLIT
        wstart = np.searchsorted(d_c, np.arange(wpc) * 128)
        wend = np.searchsorted(d_c, np.arange(1, wpc + 1) * 128)
        for w in range(wpc):
            b0 = int(woff[w])
            for grp, nbg, boff in ((0, int(nb1[w]), b0), (1, int(nb2[w]), b0 + int(nb1[w]))):
                if nbg == 0:
                    continue
                m = slice(wstart[w], wend[w])
                sel = g2[m] if grp else ~g2[m]
                sg = sr[m][sel] - (SPLIT if grp else 0)
                dg = d_c[m][sel]
                cntg = len(sg)
                idx = np.zeros(nbg * 128, dtype=np.int16)
                idx[:cntg] = sg.astype(np.int16)
                idxm[c, :, boff * 8:(boff + nbg) * 8] = _wrap16(idx)
                dloc = np.zeros(nbg * 128, dtype=np.int16)
                dloc[:cntg] = dg.astype(np.int16)  # local dst row in [0, npad)
                idxd[c, :, boff * 8:(boff + nbg) * 8] = _wrap16(dloc)
                # dstloc in [p, b] slot layout: slot i=(j*128+p) at [p, boff+j]
                # slot-linear order i = j*128+p (dma_gather writes slot i to
                # [i%128, i//128]); the flat buffer is [b][p]-ordered, which
                # the final transpose below turns into the [p, b] layout.
                dl = np.full(nbg * 128, PADLOC, dtype=np.float32)
                dl[:cntg] = (dg - w * 128).astype(np.float32)
                dstloc[c, boff * 128:(boff + nbg) * 128] = dl
                sv = np.zeros(nbg * 128, dtype=np.int32)
                sv[:cntg] = (sg.astype(np.int64) + (SPLIT if grp else 0)).astype(np.int32)
                six[c, boff * 128:(boff + nbg) * 128] = sv
                dv = np.zeros(nbg * 128, dtype=np.int32)
                dv[:cntg] = dg.astype(np.int32)
                dix[c, boff * 128:(boff + nbg) * 128] = dv
    dstloc = np.ascontiguousarray(
        dstloc.reshape(NCORES, btot, 128).transpose(0, 2, 1)).astype(np.float32)
    six = np.ascontiguousarray(six.reshape(NCORES, btot, 128).transpose(0, 2, 1))
    dix = np.ascontiguousarray(dix.reshape(NCORES, btot, 128).transpose(0, 2, 1))

    # fused GEMM weights, head-interleaved columns c = f*HEADS + h
    W1f = W1.reshape(EMB, HEADS, FDIM).transpose(0, 2, 1).reshape(EMB, EMB)
    A_s1 = np.einsum("dhf,hf->dh", W1.reshape(EMB, HEADS, FDIM), a_src1)
    A_d1 = np.einsum("dhf,hf->dh", W1.reshape(EMB, HEADS, FDIM), a_dst1)
    wc1 = np.zeros((EMB, DG), dtype=np.float32)
    wc1[:, :EMB] = W1f
    wc1[:, EMB:EMB + HEADS] = A_s1
    wc1[:, 272:280] = A_d1
    brow1 = np.zeros((DG,), dtype=np.float32)
    brow1[:EMB] = b1.reshape(HEADS, FDIM).T.reshape(EMB)  # interleaved
    brow1[264:272] = 1.0  # ones columns -> z via the aggregation matmul

    c_idx = np.arange(EMB)
    perm = (c_idx % HEADS) * FDIM + (c_idx // HEADS)
    W2p = W2.reshape(EMB, EMB)[perm, :]
    wc2 = np.zeros((EMB, DG), dtype=np.float32)
    wc2[:, :EMB] = W2p
    wc2[:, EMB] = W2p @ a_src2.reshape(EMB)
    wc2[:, 272] = W2p @ a_dst2.reshape(EMB)
    brow2 = np.zeros((DG,), dtype=np.float32)
    brow2[:EMB] = b2
    brow2[257] = 1.0  # ones column for layer-2 z

    xsT = np.zeros((NCORES, EMB, npad), dtype=np.float32)
    for c in range(NCORES):
        xsT[c, :, :nloc] = x[c * nloc:(c + 1) * nloc].T

    common = {
        "wc1": wc1.astype(ml_dtypes.bfloat16),
        "wc2": wc2.astype(ml_dtypes.bfloat16),
        "brow1": brow1.astype(ml_dtypes.bfloat16)[None, :],
        "brow2": brow2.astype(ml_dtypes.bfloat16)[None, :],
    }
    in_maps = []
    for c in range(NCORES):
        in_maps.append(dict(common,
                            xsT=xsT[c].astype(ml_dtypes.bfloat16),
                            idxm=idxm[c], idxd=idxd[c],
                            six=six[c], dix=dix[c],
                            dstloc=dstloc[c]))
    meta = dict(n=n, nloc=nloc, npad=npad, wpc=wpc,
                nb1=[int(v) for v in nb1], nb2=[int(v) for v in nb2],
                btot=btot)
    return in_maps, meta


def build(meta):
    npad, wpc, btot = meta["npad"], meta["wpc"], meta["btot"]
    nb1, nb2 = meta["nb1"], meta["nb2"]
    ntot = npad * NCORES
    nc = bacc.Bacc("TRN2", target_bir_lowering=False, debug=False, num_devices=NCORES)

    xsT = nc.dram_tensor("xsT", [EMB, npad], BF16, kind="ExternalInput")
    wc1 = nc.dram_tensor("wc1", [EMB, DG], BF16, kind="ExternalInput")
    wc2 = nc.dram_tensor("wc2", [EMB, DG], BF16, kind="ExternalInput")
    brow1 = nc.dram_tensor("brow1", [1, DG], BF16, kind="ExternalInput")
    brow2 = nc.dram_tensor("brow2", [1, DG], BF16, kind="ExternalInput")
    idxm = nc.dram_tensor("idxm", [128, btot * 8], I16, kind="ExternalInput")
    idxd = nc.dram_tensor("idxd", [128, btot * 8], I16, kind="ExternalInput")
    dstloc = nc.dram_tensor("dstloc", [128, btot], F32, kind="ExternalInput")
    six = nc.dram_tensor("six", [128, btot], mybir.dt.int32, kind="ExternalInput")
    dix = nc.dram_tensor("dix", [128, btot], mybir.dt.int32, kind="ExternalInput")
    out = nc.dram_tensor("out", [npad, EMB], F32, kind="ExternalOutput")

    t1loc = nc.dram_tensor("t1loc", [npad, DT], BF16)
    t2loc = nc.dram_tensor("t2loc", [npad, DT], BF16)
    ald1 = nc.dram_tensor("ald1", [npad, ADC], F32)
    ald2 = nc.dram_tensor("ald2", [npad, ADC], F32)
    t1 = nc.dram_tensor("t1", [ntot, DT], BF16, addr_space="Shared")
    t2 = nc.dram_tensor("t2", [ntot, DT], BF16, addr_space="Shared")

    rg = [list(range(NCORES))]

    with tile.TileContext(nc) as tc:
        with (
            tc.tile_pool(name="const", bufs=1) as constp,
            tc.tile_pool(name="psum", bufs=2, space="PSUM") as psump,
        ):
            iota_i = constp.tile([128, 128], mybir.dt.int32)
            nc.gpsimd.iota(iota_i[:], pattern=[[1, 128]], base=0, channel_multiplier=0)
            iota128 = constp.tile([128, 128], BF16)
            nc.vector.tensor_copy(out=iota128[:], in_=iota_i[:])
            ones_row = constp.tile([1, 128], BF16)
            nc.vector.memset(ones_row[:], 1.0)
            ident = constp.tile([128, 128], BF16)
            make_identity(nc, ident[:])

            idxm_t = constp.tile([128, btot * 8], I16)
            nc.sync.dma_start(out=idxm_t[:], in_=idxm[:])
            idxd_t = constp.tile([128, btot * 8], I16)
            nc.sync.dma_start(out=idxd_t[:], in_=idxd[:])
            dstloc_t = constp.tile([128, btot], F32)
            nc.sync.dma_start(out=dstloc_t[:], in_=dstloc[:])
            six_t = constp.tile([128, btot], mybir.dt.int32)
            nc.sync.dma_start(out=six_t[:], in_=six[:])
            dix_t = constp.tile([128, btot], mybir.dt.int32)
            nc.sync.dma_start(out=dix_t[:], in_=dix[:])

            o1T_0 = constp.tile([128, npad], BF16, tag="o1T0")
            o1T_1 = constp.tile([128, npad], BF16, tag="o1T1")
            o1T = [o1T_0, o1T_1]

            def node_gemm(wc_dram, brow_dram, tdst, ald_dst, heads, src_tiles=None):
                with tc.tile_pool(name="gemm", bufs=2) as gp:
                    wck = []
                    for k in range(2):
                        t = gp.tile([128, DG], BF16, tag=f"wc{k}")
                        nc.sync.dma_start(out=t[:], in_=wc_dram[k * 128:(k + 1) * 128, :])
                        wck.append(t)
                    br = gp.tile([1, DG], BF16, tag="brow")
                    nc.sync.dma_start(out=br[:], in_=brow_dram[:])
                    if src_tiles is None:
                        xk = []
                        for k in range(2):
                            t = gp.tile([128, npad], BF16, tag=f"x{k}")
                            nc.sync.dma_start(out=t[:], in_=xsT[k * 128:(k + 1) * 128, :])
                            xk.append(t)
                    else:
                        xk = src_tiles
                    for m in range(npad // 128):
                        ps = psump.tile([128, DG], F32, tag="gemm_ps", bufs=2)
                        sl = slice(m * 128, (m + 1) * 128)
                        nc.tensor.matmul(ps[:], lhsT=xk[0][:, sl], rhs=wck[0][:], start=True, stop=False)
                        nc.tensor.matmul(ps[:], lhsT=xk[1][:, sl], rhs=wck[1][:], start=False, stop=False)
                        nc.tensor.matmul(ps[:], lhsT=ones_row[:], rhs=br[:], start=False, stop=True)
                        ot = gp.tile([128, 272], BF16, tag="gemm_out", bufs=3)
                        nc.scalar.copy(out=ot[:], in_=ps[:, 0:272])
                        nc.sync.dma_start(out=tdst[sl, 0:272], in_=ot[:])
                        at = gp.tile([128, heads], F32, tag="gemm_ald", bufs=3)
                        nc.scalar.copy(out=at[:], in_=ps[:, 272:272 + heads])
                        nc.sync.dma_start(out=ald_dst[sl, 0:heads], in_=at[:])

            def edge_phase(tbl, ald_dram, heads, writer):
                fexp = 272 // 8 if heads > 1 else 0  # 34 groups of 8 cols (L1)
                zc0 = 264 if heads > 1 else 257      # z column offset in psum
                rw = 272 if heads > 1 else 258       # agg matmul rhs width
                with tc.tile_pool(name="edge", bufs=2) as ep:
                    b0 = 0
                    for w in range(wpc):
                        m1, m2 = nb1[w], nb2[w]
                        nb = m1 + m2
                        ht = ep.tile([128, nb * DT], BF16, tag="ht", bufs=2)
                        htv = ht[:].rearrange("p (j c) -> p j c", c=DT)
                        alde = ep.tile([128, nb * ADC], F32, tag="alde", bufs=2)
                        aldev = alde[:].rearrange("p (j c) -> p j c", c=ADC)
                        if USE_DMA_GATHER:
                            nc.gpsimd.dma_gather(
                                out_ap=htv[:, 0:m1, :], in_ap=tbl[0:SPLIT, :],
                                idxs_ap=idxm_t[:, b0 * 8:(b0 + m1) * 8],
                                num_idxs=m1 * 128, num_idxs_reg=m1 * 128,
                                elem_size=DT, single_packet=False)
                            if m2 > 0:
                                nc.gpsimd.dma_gather(
                                    out_ap=htv[:, m1:nb, :],
                                    in_ap=tbl[SPLIT:ntot, :],
                                    idxs_ap=idxm_t[:, (b0 + m1) * 8:(b0 + nb) * 8],
                                    num_idxs=m2 * 128, num_idxs_reg=m2 * 128,
                                    elem_size=DT, single_packet=False)
                            nc.gpsimd.dma_gather(
                                out_ap=aldev[:, :, :], in_ap=ald_dram[:],
                                idxs_ap=idxd_t[:, b0 * 8:(b0 + nb) * 8],
                                num_idxs=nb * 128, num_idxs_reg=nb * 128,
                                elem_size=ADC, single_packet=False)
                        else:
                            for j in range(nb):
                                nc.gpsimd.indirect_dma_start(
                                    out=htv[:, j, 0:272], out_offset=None, in_=tbl[:],
                                    in_offset=bass.IndirectOffsetOnAxis(
                                        ap=six_t[:, b0 + j:b0 + j + 1], axis=0))
                                nc.gpsimd.indirect_dma_start(
                                    out=aldev[:, j, 0:heads], out_offset=None, in_=ald_dram[:],
                                    in_offset=bass.IndirectOffsetOnAxis(
                                        ap=dix_t[:, b0 + j:b0 + j + 1], axis=0))
                        wpre = ep.tile([128, nb * heads], F32, tag="wpre", bufs=2)
                        nc.vector.tensor_add(
                            out=wpre[:].rearrange("p (j h) -> p j h", h=heads),
                            in0=htv[:, :, EMB:EMB + heads],
                            in1=aldev[:, :, 0:heads])
                        wlr = ep.tile([128, nb * heads], F32, tag="wlr", bufs=2)
                        nc.scalar.activation(out=wlr[:], in_=wpre[:],
                                             func=mybir.ActivationFunctionType.Prelu,
                                             alpha=NEG_SLOPE)
                        wwin = ep.tile([128, nb * heads], BF16 if heads > 1 else F32,
                                       tag="wwin", bufs=2)
                        nc.scalar.activation(out=wwin[:], in_=wlr[:],
                                             func=mybir.ActivationFunctionType.Exp)
                        if heads > 1:
                            htsc = ep.tile([128, nb * 272], BF16, tag="htsc", bufs=2)
                            nc.vector.tensor_mul(
                                out=htsc[:].rearrange("p (j f h) -> p j f h", f=fexp, h=heads),
                                in0=htv[:, :, 0:272].rearrange("p j (f h) -> p j f h", h=heads),
                                in1=wwin[:].rearrange("p (j h) -> p j h", h=heads)
                                    .unsqueeze(2).to_broadcast([128, nb, fexp, heads]))
                        ps = psump.tile([128, rw], F32, tag="agg")
                        for j in range(nb):
                            b = b0 + j
                            s_ed = ep.tile([128, 128], BF16, tag="s_ed", bufs=4)
                            if heads > 1:
                                nc.vector.tensor_scalar(
                                    out=s_ed[:], in0=iota128[:],
                                    scalar1=dstloc_t[:, b:b + 1], scalar2=None,
                                    op0=mybir.AluOpType.is_equal)
                                rhs = htsc[:, j * 272:(j + 1) * 272]
                            else:
                                nc.vector.tensor_scalar(
                                    out=s_ed[:], in0=iota128[:],
                                    scalar1=dstloc_t[:, b:b + 1],
                                    scalar2=wwin[:, j:j + 1],
                                    op0=mybir.AluOpType.is_equal,
                                    op1=mybir.AluOpType.mult)
                                rhs = ht[:, j * DT:j * DT + rw]
                            nc.tensor.matmul(ps[:], lhsT=s_ed[:], rhs=rhs,
                                             start=(j == 0), stop=(j == nb - 1))
                        zn = heads
                        zeps = ep.tile([128, zn], F32, tag="zeps", bufs=2)
                        nc.vector.tensor_scalar_add(out=zeps[:], in0=ps[:, zc0:zc0 + zn],
                                                    scalar1=1e-16)
                        rz = ep.tile([128, zn], F32, tag="rz", bufs=2)
                        nc.vector.reciprocal(out=rz[:], in_=zeps[:])
                        writer(w, ps, rz, ep)
                        b0 += nb

            # ---- layer 1 ----
            node_gemm(wc1, brow1, t1loc, ald1, HEADS)
            nc.gpsimd.collective_compute(
                "AllGather", mybir.AluOpType.bypass, replica_groups=rg,
                ins=[t1loc[:]], outs=[t1[:]])

            def write1(w, ps, rz, ep):
                ot = ep.tile([128, EMB], BF16, tag="outw", bufs=2)
                nc.vector.tensor_mul(
                    out=ot[:].rearrange("p (f h) -> p f h", h=HEADS),
                    in0=ps[:, 0:EMB].rearrange("p (f h) -> p f h", h=HEADS),
                    in1=rz[:].unsqueeze(1).to_broadcast([128, FDIM, HEADS]))
                for k in range(2):
                    pst = psump.tile([128, 128], BF16, tag="tr_ps")
                    nc.tensor.transpose(out=pst[:], in_=ot[:, k * 128:(k + 1) * 128],
                                        identity=ident[:])
                    nc.scalar.copy(out=o1T[k][:, w * 128:(w + 1) * 128], in_=pst[:])

            edge_phase(t1, ald1, HEADS, write1)

            # ---- layer 2 ----
            node_gemm(wc2, brow2, t2loc, ald2, 1, src_tiles=o1T)
            nc.gpsimd.collective_compute(
                "AllGather", mybir.AluOpType.bypass, replica_groups=rg,
                ins=[t2loc[:]], outs=[t2[:]])

            def write2(w, ps, rz, ep):
                ot = ep.tile([128, EMB], F32, tag="outw2", bufs=2)
                nc.vector.tensor_mul(
                    out=ot[:], in0=ps[:, 0:EMB],
                    in1=rz[:, 0:1].to_broadcast([128, EMB]))
                nc.sync.dma_start(out=out[w * 128:(w + 1) * 128, :], in_=ot[:])

            edge_phase(t2, ald2, 1, write2)

    nc.compile()
    return nc


def kernel(**inputs):
    inputs = {k: np.asarray(v) for k, v in inputs.items()}
    in_maps, meta = preprocess(**inputs)
    nc = build(meta)
    res = run_bass_kernel_spmd(nc, in_maps, list(range(NCORES)))
    nloc = meta["nloc"]
    parts = [res.results[c]["out"][:nloc] for c in range(NCORES)]
    return np.concatenate(parts, axis=0).astype(np.float32)


# revision 15
# speedup vs baseline: 1.8224x; 1.3293x over previous
"""2-layer GAT on 8 Trainium2 NeuronCores.

Strategy (dst-sharded, gather-based, batched via dma_gather):
- Nodes split into 8 contiguous ranges (6250/core, padded to 6272). Each core
  owns all edges whose destination lies in its range, sorted by dst, grouped
  into 49 windows of 128 dst nodes.
- Per layer: data-parallel fused node GEMM in bf16 producing table rows
  [h(256) | al_s(H) | ones(H) | pad] with 384-col (768B) stride, plus a small
  local al_d table [npad, 64] f32. Node tables are AllGathered.
- Edge phase, per 128-dst window (nb = blocks of 128 edge slots):
    * batched src-row gather via gpsimd.dma_gather (int16 indices; edges are
      regrouped per window into src-row < 32768 and >= 32768 groups since
      indices are int16; the second gather reads a rebased table slice)
    * batched per-edge al_d gather from the local table (indices < npad)
    * w = exp(leakyrelu(al_s + al_d)): one DVE add + ACT Prelu + ACT Exp
    * htsc = [h | als | ones] * w in one bf16 DVE op; the ones columns yield
      w itself, so ONE matmul per 128-edge block accumulates both the
      weighted-message sum (cols 0:256) and the softmax denominators z
      (cols 264:272): psum += S_ed^T @ htsc
    * finalize: out = psum[:, 0:256] * recip(z); layer 1 transposes straight
      into SBUF tiles feeding the layer-2 GEMM (no DRAM round trip).
- Softmax max-subtraction dropped (cancels in alpha; logits are O(1)).
- Bias folded into the table h columns (alpha sums to 1 per destination).
"""

import numpy as np
import ml_dtypes

import concourse.bass as bass
import concourse.bacc as bacc
import concourse.tile as tile
from concourse.masks import make_identity
from concourse import mybir
from concourse.bass_utils import run_bass_kernel_spmd

BF16 = mybir.dt.bfloat16
F32 = mybir.dt.float32
I16 = mybir.dt.int16

NCORES = 8
EMB = 256
HEADS = 8
FDIM = 32
NEG_SLOPE = 0.2
DT = 272          # table row stride (cols): 256 h + 8 als + 8 ones
DG = 280          # GEMM out cols: h 0:256, als 256:264, (ones via bias row
                  # 264:272), al_d 272:280
ADC = 8           # al_d table row cols (bf16), al_d in cols 0:H
SPLIT = 32768     # int16 index limit: edges grouped by src row < / >= SPLIT
PADLOC = 300.0    # dstlocal sentinel for padded edge slots (no iota match)
USE_DMA_GATHER = False  # False: per-block indirect_dma_start fallback


def _pad128(n):
    return (n + 127) // 128 * 128


def _wrap16(idx):
    """int16 index array -> [16, ceil(n/16)] wrapped layout, tiled to 128."""
    n = len(idx)
    cols = (n + 15) // 16
    buf = np.zeros(cols * 16, dtype=np.int16)
    buf[:n] = idx
    w = np.ascontiguousarray(buf.reshape(cols, 16).T)  # element i at (i%16, i//16)
    return np.tile(w, (8, 1))


def preprocess(x, edge_index, W1, a_src1, a_dst1, b1, W2, a_src2, a_dst2, b2):
    n = x.shape[0]
    nloc = n // NCORES
    assert nloc * NCORES == n
    npad = _pad128(nloc)
    wpc = npad // 128

    src = np.concatenate([edge_index[0], np.arange(n, dtype=np.int64)]).astype(np.int64)
    dst = np.concatenate([edge_index[1], np.arange(n, dtype=np.int64)]).astype(np.int64)
    order = np.argsort(dst, kind="stable")
    src_s = src[order].astype(np.int64)
    dst_s = dst[order].astype(np.int64)
    srcrow = (src_s // nloc) * npad + (src_s % nloc)  # padded global table row

    bounds = np.searchsorted(dst_s, np.arange(NCORES + 1) * nloc)
    # per-core per-window per-group counts
    cnt1 = np.zeros((NCORES, wpc), dtype=np.int64)
    cnt2 = np.zeros((NCORES, wpc), dtype=np.int64)
    for c in range(NCORES):
        sl = slice(bounds[c], bounds[c + 1])
        dl = dst_s[sl] - c * nloc
        g2 = srcrow[sl] >= SPLIT
        cnt1[c] = np.bincount((dl // 128)[~g2], minlength=wpc)
        cnt2[c] = np.bincount((dl // 128)[g2], minlength=wpc)
    nb1 = np.maximum(1, (cnt1.max(axis=0) + 127) // 128).astype(np.int64)
    nb2 = ((cnt2.max(axis=0) + 127) // 128).astype(np.int64)
    bw = nb1 + nb2
    btot = int(bw.sum())
    woff = np.concatenate([[0], np.cumsum(bw)])  # block offset per window

    idxm = np.zeros((NCORES, 128, btot * 8), dtype=np.int16)
    idxd = np.zeros((NCORES, 128, btot * 8), dtype=np.int16)
    six = np.zeros((NCORES, btot * 128), dtype=np.int32)
    dix = np.zeros((NCORES, btot * 128), dtype=np.int32)
    dstloc = np.full((NCORES, btot * 128), PADLOC, dtype=np.float32)
    for c in range(NCORES):
        sl = slice(bounds[c], bounds[c + 1])
        sr = srcrow[sl]
        d_c = (dst_s[sl] - c * nloc).astype(np.int64)
        g2 = sr >= SPLIT
        wstart = np.searchsorted(d_c, np.arange(wpc) * 128)
        wend = np.searchsorted(d_c, np.arange(1, wpc + 1) * 128)
        for w in range(wpc):
            b0 = int(woff[w])
            for grp, nbg, boff in ((0, int(nb1[w]), b0), (1, int(nb2[w]), b0 + int(nb1[w]))):
                if nbg == 0:
                    continue
                m = slice(wstart[w], wend[w])
                sel = g2[m] if grp else ~g2[m]
                sg = sr[m][sel] - (SPLIT if grp else 0)
                dg = d_c[m][sel]
                cntg = len(sg)
                idx = np.zeros(nbg * 128, dtype=np.int16)
                idx[:cntg] = sg.astype(np.int16)
                idxm[c, :, boff * 8:(boff + nbg) * 8] = _wrap16(idx)
                dloc = np.zeros(nbg * 128, dtype=np.int16)
                dloc[:cntg] = dg.astype(np.int16)  # local dst row in [0, npad)
                idxd[c, :, boff * 8:(boff + nbg) * 8] = _wrap16(dloc)
                # dstloc in [p, b] slot layout: slot i=(j*128+p) at [p, boff+j]
                # slot-linear order i = j*128+p (dma_gather writes slot i to
                # [i%128, i//128]); the flat buffer is [b][p]-ordered, which
                # the final transpose below turns into the [p, b] layout.
                dl = np.full(nbg * 128, PADLOC, dtype=np.float32)
                dl[:cntg] = (dg - w * 128).astype(np.float32)
                dstloc[c, boff * 128:(boff + nbg) * 128] = dl
                sv = np.zeros(nbg * 128, dtype=np.int32)
                sv[:cntg] = (sg.astype(np.int64) + (SPLIT if grp else 0)).astype(np.int32)
                six[c, boff * 128:(boff + nbg) * 128] = sv
                dv = np.zeros(nbg * 128, dtype=np.int32)
                dv[:cntg] = dg.astype(np.int32)
                dix[c, boff * 128:(boff + nbg) * 128] = dv
    dstlocr = dstloc.reshape(NCORES, btot, 128).astype(ml_dtypes.bfloat16)
    dstloc = np.ascontiguousarray(
        dstloc.reshape(NCORES, btot, 128).transpose(0, 2, 1)).astype(np.float32)
    six = np.ascontiguousarray(six.reshape(NCORES, btot, 128).transpose(0, 2, 1))
    dix = np.ascontiguousarray(dix.reshape(NCORES, btot, 128).transpose(0, 2, 1))

    # fused GEMM weights, head-interleaved columns c = f*HEADS + h
    W1f = W1.reshape(EMB, HEADS, FDIM).transpose(0, 2, 1).reshape(EMB, EMB)
    A_s1 = np.einsum("dhf,hf->dh", W1.reshape(EMB, HEADS, FDIM), a_src1)
    A_d1 = np.einsum("dhf,hf->dh", W1.reshape(EMB, HEADS, FDIM), a_dst1)
    wc1 = np.zeros((EMB, DG), dtype=np.float32)
    wc1[:, :EMB] = W1f
    wc1[:, EMB:EMB + HEADS] = A_s1
    wc1[:, 272:280] = A_d1
    brow1 = np.zeros((DG,), dtype=np.float32)
    brow1[:EMB] = b1.reshape(HEADS, FDIM).T.reshape(EMB)  # interleaved
    brow1[264:272] = 1.0  # ones columns -> z via the aggregation matmul

    c_idx = np.arange(EMB)
    perm = (c_idx % HEADS) * FDIM + (c_idx // HEADS)
    W2p = W2.reshape(EMB, EMB)[perm, :]
    wc2 = np.zeros((EMB, DG), dtype=np.float32)
    wc2[:, :EMB] = W2p
    wc2[:, EMB] = W2p @ a_src2.reshape(EMB)
    wc2[:, 272] = W2p @ a_dst2.reshape(EMB)
    brow2 = np.zeros((DG,), dtype=np.float32)
    brow2[:EMB] = b2
    brow2[257] = 1.0  # ones column for layer-2 z

    xsT = np.zeros((NCORES, EMB, npad), dtype=np.float32)
    for c in range(NCORES):
        xsT[c, :, :nloc] = x[c * nloc:(c + 1) * nloc].T

    common = {
        "wc1": wc1.astype(ml_dtypes.bfloat16),
        "wc2": wc2.astype(ml_dtypes.bfloat16),
        "brow1": brow1.astype(ml_dtypes.bfloat16)[None, :],
        "brow2": brow2.astype(ml_dtypes.bfloat16)[None, :],
    }
    in_maps = []
    for c in range(NCORES):
        in_maps.append(dict(common,
                            xsT=xsT[c].astype(ml_dtypes.bfloat16),
                            idxm=idxm[c], idxd=idxd[c],
                            six=six[c], dix=dix[c],
                            dstlocr=dstlocr[c],
                            dstloc=dstloc[c]))
    meta = dict(n=n, nloc=nloc, npad=npad, wpc=wpc,
                nb1=[int(v) for v in nb1], nb2=[int(v) for v in nb2],
                btot=btot)
    return in_maps, meta


def build(meta):
    npad, wpc, btot = meta["npad"], meta["wpc"], meta["btot"]
    nb1, nb2 = meta["nb1"], meta["nb2"]
    ntot = npad * NCORES
    nc = bacc.Bacc("TRN2", target_bir_lowering=False, debug=False, num_devices=NCORES)

    xsT = nc.dram_tensor("xsT", [EMB, npad], BF16, kind="ExternalInput")
    wc1 = nc.dram_tensor("wc1", [EMB, DG], BF16, kind="ExternalInput")
    wc2 = nc.dram_tensor("wc2", [EMB, DG], BF16, kind="ExternalInput")
    brow1 = nc.dram_tensor("brow1", [1, DG], BF16, kind="ExternalInput")
    brow2 = nc.dram_tensor("brow2", [1, DG], BF16, kind="ExternalInput")
    idxm = nc.dram_tensor("idxm", [128, btot * 8], I16, kind="ExternalInput")
    idxd = nc.dram_tensor("idxd", [128, btot * 8], I16, kind="ExternalInput")
    dstloc = nc.dram_tensor("dstloc", [128, btot], F32, kind="ExternalInput")
    six = nc.dram_tensor("six", [128, btot], mybir.dt.int32, kind="ExternalInput")
    dstlocr = nc.dram_tensor("dstlocr", [btot, 128], BF16, kind="ExternalInput")
    dix = nc.dram_tensor("dix", [128, btot], mybir.dt.int32, kind="ExternalInput")
    out = nc.dram_tensor("out", [npad, EMB], F32, kind="ExternalOutput")

    t1loc = nc.dram_tensor("t1loc", [npad, DT], BF16)
    t2loc = nc.dram_tensor("t2loc", [npad, DT], BF16)
    ald1 = nc.dram_tensor("ald1", [npad, ADC], BF16)
    ald2 = nc.dram_tensor("ald2", [npad, ADC], BF16)
    t1 = nc.dram_tensor("t1", [ntot, DT], BF16, addr_space="Shared")
    t2 = nc.dram_tensor("t2", [ntot, DT], BF16, addr_space="Shared")

    rg = [list(range(NCORES))]

    with tile.TileContext(nc) as tc:
        with (
            tc.tile_pool(name="const", bufs=1) as constp,
            tc.tile_pool(name="psum", bufs=2, space="PSUM") as psump,
        ):
            iota_i = constp.tile([128, 128], mybir.dt.int32)
            nc.gpsimd.iota(iota_i[:], pattern=[[1, 128]], base=0, channel_multiplier=0)
            iota128 = constp.tile([128, 128], BF16)
            nc.vector.tensor_copy(out=iota128[:], in_=iota_i[:])
            ones_row = constp.tile([1, 128], BF16)
            nc.vector.memset(ones_row[:], 1.0)
            iotac_i = constp.tile([128, 1], mybir.dt.int32)
            nc.gpsimd.iota(iotac_i[:], pattern=[[0, 1]], base=0, channel_multiplier=1)
            iota_col = constp.tile([128, 1], F32)
            nc.vector.tensor_copy(out=iota_col[:], in_=iotac_i[:])
            ident = constp.tile([128, 128], BF16)
            make_identity(nc, ident[:])

            idxm_t = constp.tile([128, btot * 8], I16)
            nc.sync.dma_start(out=idxm_t[:], in_=idxm[:])
            idxd_t = constp.tile([128, btot * 8], I16)
            nc.sync.dma_start(out=idxd_t[:], in_=idxd[:])
            dstloc_t = constp.tile([128, btot], F32)
            nc.sync.dma_start(out=dstloc_t[:], in_=dstloc[:])
            six_t = constp.tile([128, btot], mybir.dt.int32)
            nc.sync.dma_start(out=six_t[:], in_=six[:])
            dix_t = constp.tile([128, btot], mybir.dt.int32)
            nc.sync.dma_start(out=dix_t[:], in_=dix[:])

            o1T_0 = constp.tile([128, npad], BF16, tag="o1T0")
            o1T_1 = constp.tile([128, npad], BF16, tag="o1T1")
            o1T = [o1T_0, o1T_1]

            def node_gemm(wc_dram, brow_dram, tdst, ald_dst, heads, src_tiles=None):
                with tc.tile_pool(name="gemm", bufs=2) as gp:
                    wck = []
                    for k in range(2):
                        t = gp.tile([128, DG], BF16, tag=f"wc{k}")
                        nc.sync.dma_start(out=t[:], in_=wc_dram[k * 128:(k + 1) * 128, :])
                        wck.append(t)
                    br = gp.tile([1, DG], BF16, tag="brow")
                    nc.sync.dma_start(out=br[:], in_=brow_dram[:])
                    if src_tiles is None:
                        xk = []
                        for k in range(2):
                            t = gp.tile([128, npad], BF16, tag=f"x{k}")
                            nc.sync.dma_start(out=t[:], in_=xsT[k * 128:(k + 1) * 128, :])
                            xk.append(t)
                    else:
                        xk = src_tiles
                    for m in range(npad // 128):
                        ps = psump.tile([128, DG], F32, tag="gemm_ps", bufs=1)
                        sl = slice(m * 128, (m + 1) * 128)
                        nc.tensor.matmul(ps[:], lhsT=xk[0][:, sl], rhs=wck[0][:], start=True, stop=False)
                        nc.tensor.matmul(ps[:], lhsT=xk[1][:, sl], rhs=wck[1][:], start=False, stop=False)
                        nc.tensor.matmul(ps[:], lhsT=ones_row[:], rhs=br[:], start=False, stop=True)
                        ot = gp.tile([128, 272], BF16, tag="gemm_out", bufs=3)
                        nc.scalar.copy(out=ot[:], in_=ps[:, 0:272])
                        nc.sync.dma_start(out=tdst[sl, 0:272], in_=ot[:])
                        at = gp.tile([128, heads], BF16, tag="gemm_ald", bufs=3)
                        nc.scalar.copy(out=at[:], in_=ps[:, 272:272 + heads])
                        nc.sync.dma_start(out=ald_dst[sl, 0:heads], in_=at[:])

            def edge_phase(tbl, ald_dram, heads, writer):
                fexp = 272 // 8 if heads > 1 else 0  # 34 groups of 8 cols (L1)
                zc0 = 264 if heads > 1 else 257      # z column offset in psum
                rw = 272 if heads > 1 else 258       # agg matmul rhs width
                with tc.tile_pool(name="edge", bufs=2) as ep:
                    b0 = 0
                    for w in range(wpc):
                        m1, m2 = nb1[w], nb2[w]
                        nb = m1 + m2
                        ht = ep.tile([128, nb * DT], BF16, tag="ht", bufs=2)
                        htv = ht[:].rearrange("p (j c) -> p j c", c=DT)
                        if USE_DMA_GATHER:
                            alde = ep.tile([128, nb * ADC], F32, tag="alde", bufs=2)
                            aldev = alde[:].rearrange("p (j c) -> p j c", c=ADC)
                            nc.gpsimd.dma_gather(
                                out_ap=htv[:, 0:m1, :], in_ap=tbl[0:SPLIT, :],
                                idxs_ap=idxm_t[:, b0 * 8:(b0 + m1) * 8],
                                num_idxs=m1 * 128, num_idxs_reg=m1 * 128,
                                elem_size=DT, single_packet=False)
                            if m2 > 0:
                                nc.gpsimd.dma_gather(
                                    out_ap=htv[:, m1:nb, :],
                                    in_ap=tbl[SPLIT:ntot, :],
                                    idxs_ap=idxm_t[:, (b0 + m1) * 8:(b0 + nb) * 8],
                                    num_idxs=m2 * 128, num_idxs_reg=m2 * 128,
                                    elem_size=DT, single_packet=False)
                            nc.gpsimd.dma_gather(
                                out_ap=aldev[:, :, :], in_ap=ald_dram[:],
                                idxs_ap=idxd_t[:, b0 * 8:(b0 + nb) * 8],
                                num_idxs=nb * 128, num_idxs_reg=nb * 128,
                                elem_size=ADC, single_packet=False)
                        else:
                            for j in range(nb):
                                nc.gpsimd.indirect_dma_start(
                                    out=htv[:, j, 0:272], out_offset=None, in_=tbl[:],
                                    in_offset=bass.IndirectOffsetOnAxis(
                                        ap=six_t[:, b0 + j:b0 + j + 1], axis=0))
                            # al_d expansion on PE: aldw (local window rows) via
                            # one-hot s_de built from the row-replicated dstloc
                            aldw = ep.tile([128, heads], BF16, tag="aldw", bufs=2)
                            nc.sync.dma_start(
                                out=aldw[:],
                                in_=ald_dram[w * 128:(w + 1) * 128, 0:heads])
                            dlr = ep.tile([128, nb * 128], BF16, tag="dlr", bufs=2)
                            nc.sync.dma_start(
                                out=dlr[:],
                                in_=dstlocr[b0:b0 + nb, :].rearrange("a b -> (a b)")
                                    .unsqueeze(0).to_broadcast([128, nb * 128]))
                            s_de = ep.tile([128, nb * 128], BF16, tag="s_de", bufs=2)
                            nc.vector.tensor_scalar(
                                out=s_de[:], in0=dlr[:],
                                scalar1=iota_col[:], scalar2=None,
                                op0=mybir.AluOpType.is_equal)
                            ps_ald = psump.tile([128, max(8, nb * heads)], F32, tag="ps_ald")
                            for j in range(nb):
                                nc.tensor.matmul(
                                    ps_ald[:, j * heads:(j + 1) * heads],
                                    lhsT=s_de[:, j * 128:(j + 1) * 128], rhs=aldw[:],
                                    start=True, stop=True)
                        wpre = ep.tile([128, nb * heads], F32, tag="wpre", bufs=2)
                        nc.vector.tensor_add(
                            out=wpre[:].rearrange("p (j h) -> p j h", h=heads),
                            in0=htv[:, :, EMB:EMB + heads],
                            in1=(aldev[:, :, 0:heads] if USE_DMA_GATHER else
                                 ps_ald[:, 0:nb * heads]
                                 .rearrange("p (j h) -> p j h", h=heads)))
                        wlr = ep.tile([128, nb * heads], F32, tag="wlr", bufs=2)
                        nc.scalar.activation(out=wlr[:], in_=wpre[:],
                                             func=mybir.ActivationFunctionType.Prelu,
                                             alpha=NEG_SLOPE)
                        wwin = ep.tile([128, nb * heads], BF16 if heads > 1 else F32,
                                       tag="wwin", bufs=2)
                        nc.scalar.activation(out=wwin[:], in_=wlr[:],
                                             func=mybir.ActivationFunctionType.Exp)
                        if heads > 1:
                            htsc = ep.tile([128, nb * 272], BF16, tag="htsc", bufs=2)
                            nc.vector.tensor_mul(
                                out=htsc[:].rearrange("p (j f h) -> p j f h", f=fexp, h=heads),
                                in0=htv[:, :, 0:272].rearrange("p j (f h) -> p j f h", h=heads),
                                in1=wwin[:].rearrange("p (j h) -> p j h", h=heads)
                                    .unsqueeze(2).to_broadcast([128, nb, fexp, heads]))
                        ps = psump.tile([128, rw], F32, tag="agg")
                        for j in range(nb):
                            b = b0 + j
                            s_ed = ep.tile([128, 128], BF16, tag="s_ed", bufs=4)
                            if heads > 1:
                                nc.vector.tensor_scalar(
                                    out=s_ed[:], in0=iota128[:],
                                    scalar1=dstloc_t[:, b:b + 1], scalar2=None,
                                    op0=mybir.AluOpType.is_equal)
                                rhs = htsc[:, j * 272:(j + 1) * 272]
                            else:
                                nc.vector.tensor_scalar(
                                    out=s_ed[:], in0=iota128[:],
                                    scalar1=dstloc_t[:, b:b + 1],
                                    scalar2=wwin[:, j:j + 1],
                                    op0=mybir.AluOpType.is_equal,
                                    op1=mybir.AluOpType.mult)
                                rhs = ht[:, j * DT:j * DT + rw]
                            nc.tensor.matmul(ps[:], lhsT=s_ed[:], rhs=rhs,
                                             start=(j == 0), stop=(j == nb - 1))
                        zn = heads
                        zeps = ep.tile([128, zn], F32, tag="zeps", bufs=2)
                        nc.vector.tensor_scalar_add(out=zeps[:], in0=ps[:, zc0:zc0 + zn],
                                                    scalar1=1e-16)
                        rz = ep.tile([128, zn], F32, tag="rz", bufs=2)
                        nc.vector.reciprocal(out=rz[:], in_=zeps[:])
                        writer(w, ps, rz, ep)
                        b0 += nb

            # ---- layer 1 ----
            node_gemm(wc1, brow1, t1loc, ald1, HEADS)
            nc.gpsimd.collective_compute(
                "AllGather", mybir.AluOpType.bypass, replica_groups=rg,
                ins=[t1loc[:]], outs=[t1[:]])

            def write1(w, ps, rz, ep):
                ot = ep.tile([128, EMB], BF16, tag="outw", bufs=2)
                nc.vector.tensor_mul(
                    out=ot[:].rearrange("p (f h) -> p f h", h=HEADS),
                    in0=ps[:, 0:EMB].rearrange("p (f h) -> p f h", h=HEADS),
                    in1=rz[:].unsqueeze(1).to_broadcast([128, FDIM, HEADS]))
                for k in range(2):
                    pst = psump.tile([128, 128], BF16, tag="tr_ps", bufs=1)
                    nc.tensor.transpose(out=pst[:], in_=ot[:, k * 128:(k + 1) * 128],
                                        identity=ident[:])
                    nc.scalar.copy(out=o1T[k][:, w * 128:(w + 1) * 128], in_=pst[:])

            edge_phase(t1, ald1, HEADS, write1)

            # ---- layer 2 ----
            node_gemm(wc2, brow2, t2loc, ald2, 1, src_tiles=o1T)
            nc.gpsimd.collective_compute(
                "AllGather", mybir.AluOpType.bypass, replica_groups=rg,
                ins=[t2loc[:]], outs=[t2[:]])

            def write2(w, ps, rz, ep):
                ot = ep.tile([128, EMB], F32, tag="outw2", bufs=2)
                nc.vector.tensor_mul(
                    out=ot[:], in0=ps[:, 0:EMB],
                    in1=rz[:, 0:1].to_broadcast([128, EMB]))
                nc.sync.dma_start(out=out[w * 128:(w + 1) * 128, :], in_=ot[:])

            edge_phase(t2, ald2, 1, write2)

    nc.compile()
    return nc


def kernel(**inputs):
    inputs = {k: np.asarray(v) for k, v in inputs.items()}
    in_maps, meta = preprocess(**inputs)
    nc = build(meta)
    res = run_bass_kernel_spmd(nc, in_maps, list(range(NCORES)))
    nloc = meta["nloc"]
    parts = [res.results[c]["out"][:nloc] for c in range(NCORES)]
    return np.concatenate(parts, axis=0).astype(np.float32)
